# revision 7
# baseline (speedup 1.0000x reference)
"""GCN (2-layer graph convolution) on 8 TRN2 NeuronCores.

Strategy (1D graph partition):
  - Nodes sharded across 8 cores (12500 rows each); edges partitioned by
    destination row so segment_sum is core-local.
  - Layer 1: each core computes support1 = x_k @ W1 (bf16); per-quarter
    AllGathers (fired as soon as each quarter is produced) build the full
    table T1 [100000, 128] bf16, 4 segments of 25000 (int16 gather idx).
  - SpMM via dma_gather of 256B rows + selection-matrix matmul
    segment-sum. Gathers are MERGED per (4-tile group, segment) and issued
    PREPARE_ONLY + trigger_dma: desc-gen (~1us+0.34ns/desc on gpsimd) runs
    ahead without blocking on the AllGather or the transfer; only the tiny
    trigger waits, and consumers wait on the DMA-completion semaphore.
    All pad slots gather row 0 (no negative indices mid-stream, no NaNs).
  - The selection matrices S[e,d] = val*(row==d) are PRECOMPUTED ON HOST
    in fp8e3m4 and STREAMED from HBM (one contiguous [128, ct*128] load
    per dst tile, chunks in tile-major order). On-chip S-building cost
    ~1ns/elem on DVE (~1ms/phase) -- streaming 57.6MB/phase instead rides
    the otherwise idle DMA headroom. PE accepts the fp8 rhs against bf16
    gathered rows (only fp32 pairing is restricted).
  - Stores batched per group with quarter-boundary splitting; phase D
    log_softmax is batched per group so Exp/Ln ACT tables load once per
    group instead of once per tile.
"""

import sys

sys.path.insert(0, "/opt/trn_rl_repo")

import numpy as np
import ml_dtypes

import concourse.bass as bass
import concourse.tile as tile
from concourse import bacc, mybir
from concourse.bass_utils import run_bass_kernel_spmd
from concourse.library_config import mlp

N = 100000
E = 3200000
F_IN, F_HID, F_OUT = 512, 128, 32
NC = 8
SHARD = N // NC          # 12500
P = 128
NT = (SHARD + P - 1) // P   # 98 tiles; last has 84 rows
NSEG = 4
SEG = N // NSEG          # 25000 rows per gather segment (fits int16 indices)
QSEG = SHARD // NSEG     # 3125: per-core sub-shard contributed to segment s
GRP = 4                  # dst tiles per merged gather / store batch
NGRP = (NT + GRP - 1) // GRP   # 25
GA = 8                   # phase-A load batch (tiles)
BF16 = ml_dtypes.bfloat16
F8 = ml_dtypes.float8_e3m4


def _preprocess(edge_row, edge_col, edge_val):
    """Sort/pad edges into per-(core, dst-tile, col-segment) buckets of
    whole 128-edge chunks (chunk counts identical across cores). Returns
    the idx stream (gather order (g,s,u,c)) and the host-built S matrices
    (tile-major order (g,u,s,c)) in fp8."""
    er = edge_row.astype(np.int64)
    ec = edge_col.astype(np.int64)
    k = er // SHARD
    t = (er % SHARD) // P
    r = ec % SHARD
    s = r // QSEG
    idx_seg = (ec // SHARD) * QSEG + (ec % QSEG)   # row within segment s
    g = t // GRP
    u = t % GRP
    key = ((k * NGRP + g) * NSEG + s) * GRP + u
    order = np.argsort(key, kind="stable")
    counts = np.bincount(key, minlength=NC * NGRP * NSEG * GRP) \
        .reshape(NC, NGRP, NSEG, GRP)
    C_gsu = -(-counts.max(axis=0) // P)         # [NGRP, NSEG, GRP]
    TC = int(C_gsu.sum())
    off_flat = np.concatenate([[0], np.cumsum(C_gsu.flatten())])[:-1]
    off_gsu = off_flat.reshape(NGRP, NSEG, GRP)

    starts = np.zeros(NC * NGRP * NSEG * GRP + 1, np.int64)
    starts[1:] = np.cumsum(counts.flatten())
    key_s = key[order]
    rank = np.arange(E, dtype=np.int64) - starts[key_s]
    base_k = (np.arange(NC)[:, None, None, None] * TC
              + off_gsu[None]) * P                  # [NC, NGRP, NSEG, GRP]
    slot = base_k.reshape(-1)[key_s] + rank

    # pad slots gather row 0 with val 0 (real DMA packet, S row all-zero)
    idx_slots = np.zeros(NC * TC * P, np.int16)
    rows_slots = np.zeros(NC * TC * P, np.int64)
    vals_slots = np.zeros(NC * TC * P, np.float32)
    ero = edge_row[order]
    idx_slots[slot] = idx_seg[order].astype(np.int16)
    rows_slots[slot] = (ero % SHARD) % P
    vals_slots[slot] = edge_val[order].astype(np.float32)

    idx_k = idx_slots.reshape(NC, TC, P)
    rows_k = rows_slots.reshape(NC, TC, P)
    vals_k = vals_slots.reshape(NC, TC, P)

    # idx stream: per (g, s) block of 8*GC i16 (16-partition wrap, 8x repl)
    GC_gs = C_gsu.sum(axis=2)                      # [NGRP, NSEG]
    moff = np.zeros((NGRP, NSEG), np.int64)
    w = 0
    for gi in range(NGRP):
        for si in range(NSEG):
            moff[gi, si] = w
            w += 8 * int(GC_gs[gi, si])
    MW = w
    meta = np.zeros((NC, P, MW), np.int16)
    for c in range(NC):
        for gi in range(NGRP):
            for si in range(NSEG):
                gc = int(GC_gs[gi, si])
                if gc == 0:
                    continue
                goff = int(off_gsu[gi, si, 0])
                o = int(moff[gi, si])
                blk = idx_k[c, goff:goff + gc].reshape(-1, 16).T  # [16, 8*gc]
                meta[c, :, o:o + 8 * gc] = np.tile(blk, (8, 1))

    # chunk permutation: tile-major position -> slot-order chunk index
    perm = np.zeros(TC, np.int64)
    toff_t = np.zeros(NT + 1, np.int64)
    q = 0
    for gi in range(NGRP):
        for ui in range(GRP):
            ti = gi * GRP + ui
            if ti >= NT:
                continue
            toff_t[ti] = q
            for si in range(NSEG):
                cs = int(C_gsu[gi, si, ui])
                o = int(off_gsu[gi, si, ui])
                perm[q:q + cs] = np.arange(o, o + cs)
                q += cs
    toff_t[NT] = q
    assert q == TC

    # host-built S: sdat[c][p, qq*128 + rows] = vals  (tile-major chunk qq)
    sdat = np.zeros((NC, P, TC * 128), F8)
    qq = np.arange(TC, dtype=np.int64)[:, None]
    pp = np.arange(P, dtype=np.int64)[None, :]
    for c in range(NC):
        rT = rows_k[c][perm]                        # [TC, P]
        vT = vals_k[c][perm].astype(F8)
        flat = pp * (TC * 128) + qq * 128 + rT      # [TC, P]
        sc = np.zeros(P * TC * 128, F8)
        sc[flat.ravel()] = vT.ravel()
        sdat[c] = sc.reshape(P, TC * 128)

    return C_gsu, off_gsu, GC_gs, moff, TC, MW, toff_t, meta, sdat


def _build_program(C_gsu, off_gsu, GC_gs, moff, TC, MW, toff_t):
    f32, bf16, i16 = mybir.dt.float32, mybir.dt.bfloat16, mybir.dt.int16
    f8 = mybir.dt.float8e3
    nc = bacc.Bacc("TRN2", target_bir_lowering=False, debug=False,
                   num_devices=NC, num_swdge_queues=4)

    xT = nc.dram_tensor("xT", [F_IN, SHARD], bf16, kind="ExternalInput")
    W1b = nc.dram_tensor("W1b", [F_IN, F_HID], bf16, kind="ExternalInput")
    W2b = nc.dram_tensor("W2b", [F_HID, F_OUT], bf16, kind="ExternalInput")
    b1c = nc.dram_tensor("b1c", [P, 1], f32, kind="ExternalInput")
    b2bc = nc.dram_tensor("b2bc", [P, F_OUT], f32, kind="ExternalInput")
    metad = nc.dram_tensor("metad", [P, MW], i16, kind="ExternalInput")
    sdat = nc.dram_tensor("sdat", [P, TC * 128], f8, kind="ExternalInput")
    out = nc.dram_tensor("out", [SHARD, F_OUT], f32, kind="ExternalOutput")

    T1q = [nc.dram_tensor(f"T1q{s}", [QSEG, F_HID], bf16)
           for s in range(NSEG)]
    T1_seg = [nc.dram_tensor(f"T1_seg{s}", [SEG, F_HID], bf16,
                             addr_space="Shared") for s in range(NSEG)]
    T2q = [nc.dram_tensor(f"T2q{s}", [QSEG, P], bf16)
           for s in range(NSEG)]
    T2_seg = [nc.dram_tensor(f"T2_seg{s}", [SEG, P], bf16,
                             addr_space="Shared") for s in range(NSEG)]

    C_t = C_gsu.sum(axis=1)          # [NGRP, GRP] chunks per tile
    CTM = int(C_t.max())             # max chunks per tile
    GCMAX = int(GC_gs.max())         # max chunks per (group, seg)
    MGMAX = int(max(
        (moff[gi + 1, 0] if gi + 1 < NGRP else MW) - moff[gi, 0]
        for gi in range(NGRP)))

    with tile.TileContext(nc) as tc:
        with (
            tc.tile_pool(name="const", bufs=1) as cpool,
            tc.tile_pool(name="xa", bufs=2) as xapool,
            tc.tile_pool(name="s1o", bufs=2) as s1pool,
            tc.tile_pool(name="g", bufs=8) as gpool,
            tc.tile_pool(name="sm", bufs=6) as smpool,
            tc.tile_pool(name="meta", bufs=4) as mpool,
            tc.tile_pool(name="ep", bufs=4) as eppool,
            tc.tile_pool(name="pa", bufs=2, space="PSUM") as papool,
            tc.tile_pool(name="ph", bufs=2, space="PSUM") as phpool,
            tc.tile_pool(name="ps", bufs=2, space="PSUM") as pspool,
        ):
            nc.gpsimd.load_library(mlp)

            # ---- resident constants ----
            W1_sb = cpool.tile([P, 4, F_HID], bf16)
            nc.sync.dma_start(
                W1_sb[:], W1b.ap().rearrange("(kk p) f -> p kk f", p=P))
            W2_sb = cpool.tile([P, F_OUT], bf16)
            nc.sync.dma_start(W2_sb[:], W2b.ap())
            b1_sb = cpool.tile([P, 1], f32)
            nc.sync.dma_start(b1_sb[:], b1c.ap())
            b2_sb = cpool.tile([P, F_OUT], f32)
            nc.sync.dma_start(b2_sb[:], b2bc.ap())

            def _store_batch(dsts, nrows_dst, src, b0, bw, width):
                """Store src [P, u, width] rows b0..b0+bw into quarter-split
                dram tensors dsts (each nrows_dst rows)."""
                r = b0
                while r < b0 + bw:
                    s = r // nrows_dst
                    r1 = min((s + 1) * nrows_dst, b0 + bw)
                    q = r
                    while q < r1:
                        u = (q - b0) // P
                        q1 = min(b0 + (u + 1) * P, r1)
                        p0 = q - b0 - u * P
                        p1 = q1 - b0 - u * P
                        if p0 == 0 and p1 == P and q1 - q >= P:
                            uend = u
                            while (q + (uend - u + 1) * P <= r1):
                                uend += 1
                            nc.sync.dma_start(
                                dsts[s].ap()[q - s * nrows_dst:
                                             q - s * nrows_dst
                                             + (uend - u) * P, :width]
                                .rearrange("(u p) f -> p u f", p=P),
                                src[:, u:uend, :width])
                            q += (uend - u) * P
                        else:
                            nc.sync.dma_start(
                                dsts[s].ap()[q - s * nrows_dst:
                                             q1 - s * nrows_dst, :width],
                                src[p0:p1, u, :width])
                            q = q1
                    r = r1

            # ---- phase A: support1 = x_k @ W1 -> T1q quarters (bf16) ----
            ag1_done = [False] * NSEG
            for g0 in range(0, NT, GA):
                gn = min(GA, NT - g0)
                b0 = g0 * P
                bw = min(GA * P, SHARD - b0)
                xa = xapool.tile([P, 4, GA * P], bf16)
                nc.sync.dma_start(
                    xa[:, :, :bw],
                    xT.ap()[:, b0:b0 + bw].rearrange("(kk p) m -> p kk m", p=P))
                s1g = s1pool.tile([P, GA, F_HID], bf16)
                for m in range(g0, g0 + gn):
                    m0 = m * P
                    mw = min(P, SHARD - m0)
                    lo = (m - g0) * P
                    ps = papool.tile([P, F_HID], f32, space="PSUM")
                    for kk in range(4):
                        nc.tensor.matmul(ps[:mw, :], xa[:, kk, lo:lo + mw],
                                         W1_sb[:, kk, :],
                                         start=(kk == 0), stop=(kk == 3))
                    nc.scalar.activation(s1g[:mw, m - g0, :], ps[:mw, :],
                                         mybir.ActivationFunctionType.Copy)
                _store_batch(T1q, QSEG, s1g, b0, bw, F_HID)
                for s in range(NSEG):
                    if not ag1_done[s] and b0 + bw >= (s + 1) * QSEG:
                        nc.gpsimd.collective_compute(
                            "AllGather", mybir.AluOpType.bypass,
                            replica_groups=[list(range(NC))],
                            ins=[T1q[s].ap().opt()],
                            outs=[T1_seg[s].ap().opt()],
                        )
                        ag1_done[s] = True

            def _load_meta(gi):
                o0 = int(moff[gi, 0])
                o1 = (int(moff[gi + 1, 0]) if gi + 1 < NGRP else MW)
                mt = mpool.tile([P, MGMAX], i16, tag="meta")
                nc.sync.dma_start(mt[:, :o1 - o0], metad.ap()[:, o0:o1])
                return mt, o0

            def _gather_group(gi, mt, o0, table, elem):
                gts = {}
                for s in range(NSEG):
                    gc = int(GC_gs[gi, s])
                    if gc == 0:
                        continue
                    o = int(moff[gi, s]) - o0
                    gt = gpool.tile([P, GCMAX, P], bf16, tag="g")
                    nc.gpsimd.dma_gather(
                        gt[:, :gc, :],
                        table[s].ap(),
                        mt[:, o:o + 8 * gc],
                        gc * P, gc * P, elem,
                        single_packet=False, queue_num=s,
                    )
                    gts[s] = gt
                return gts

            def _load_s(t):
                ct = int(C_t[t // GRP, t % GRP])
                smt = smpool.tile([P, CTM * 128], f8, tag="sm")
                nc.sync.dma_start(
                    smt[:, :ct * 128],
                    sdat.ap()[:, int(toff_t[t]) * 128:
                              (int(toff_t[t]) + ct) * 128])
                return smt, ct

            # ---- phase B: SpMM1 + Relu + @W2 -> T2q quarters ----
            ag2_done = [False] * NSEG
            for gi in range(NGRP):
                mt, o0 = _load_meta(gi)
                gts = _gather_group(gi, mt, o0, T1_seg, F_HID)
                b0 = gi * GRP * P
                bw = min(GRP * P, SHARD - b0)
                s2g = s1pool.tile([P, GRP, F_OUT], bf16, tag="s2g")
                for t in range(gi * GRP, min(NT, gi * GRP + GRP)):
                    u = t % GRP
                    smt, ct = _load_s(t)
                    ph = phpool.tile([P, P], f32, space="PSUM")
                    ci = 0
                    for s in range(NSEG):
                        cs = int(C_gsu[gi, s, u])
                        if cs == 0:
                            continue
                        lo = int(off_gsu[gi, s, u] - off_gsu[gi, s, 0])
                        for c in range(cs):
                            nc.tensor.matmul(ph[:], gts[s][:, lo + c, :],
                                             smt[:, ci * 128:(ci + 1) * 128],
                                             start=(ci == 0),
                                             stop=(ci == ct - 1))
                            ci += 1
                    hT = eppool.tile([P, P], bf16, tag="hT")
                    nc.scalar.activation(hT[:], ph[:],
                                         mybir.ActivationFunctionType.Relu,
                                         bias=b1_sb[:])
                    ps2 = pspool.tile([P, F_OUT], f32, space="PSUM")
                    nc.tensor.matmul(ps2[:], hT[:], W2_sb[:],
                                     start=True, stop=True)
                    nc.vector.tensor_copy(s2g[:, u, :], ps2[:])
                _store_batch(T2q, QSEG, s2g, b0, bw, F_OUT)
                for s in range(NSEG):
                    if not ag2_done[s] and b0 + bw >= (s + 1) * QSEG:
                        nc.gpsimd.collective_compute(
                            "AllGather", mybir.AluOpType.bypass,
                            replica_groups=[list(range(NC))],
                            ins=[T2q[s].ap().opt()],
                            outs=[T2_seg[s].ap().opt()],
                        )
                        ag2_done[s] = True

            # ---- phase D: SpMM2 + bias + log_softmax -> out ----
            for gi in range(NGRP):
                mt, o0 = _load_meta(gi)
                gts = _gather_group(gi, mt, o0, T2_seg, P)
                b0 = gi * GRP * P
                bw = min(GRP * P, SHARD - b0)
                gn = min(NT, gi * GRP + GRP) - gi * GRP
                lgg = eppool.tile([P, GRP, F_OUT], f32, tag="lgg")
                for t in range(gi * GRP, gi * GRP + gn):
                    u = t % GRP
                    smt, ct = _load_s(t)
                    pl = pspool.tile([P, F_OUT], f32, space="PSUM", tag="pl")
                    ci = 0
                    for s in range(NSEG):
                        cs = int(C_gsu[gi, s, u])
                        if cs == 0:
                            continue
                        lo = int(off_gsu[gi, s, u] - off_gsu[gi, s, 0])
                        for c in range(cs):
                            nc.tensor.matmul(pl[:],
                                             smt[:, ci * 128:(ci + 1) * 128],
                                             gts[s][:, lo + c, :F_OUT],
                                             start=(ci == 0),
                                             stop=(ci == ct - 1))
                            ci += 1
                    nc.vector.tensor_add(lgg[:, u, :], pl[:], b2_sb[:])
                nmx = eppool.tile([P, GRP, 1], f32, tag="nmx")
                nc.vector.tensor_reduce(nmx[:, :gn, :], lgg[:, :gn, :],
                                        axis=mybir.AxisListType.X,
                                        op=mybir.AluOpType.max, negate=True)
                exi = eppool.tile([P, GRP, F_OUT], f32, tag="exi")
                nc.vector.tensor_tensor(
                    exi[:, :gn, :], lgg[:, :gn, :],
                    nmx[:, :gn, :].broadcast_to([P, gn, F_OUT]),
                    op=mybir.AluOpType.add)
                ex = eppool.tile([P, GRP, F_OUT], f32, tag="ex")
                nc.scalar.activation(ex[:, :gn, :], exi[:, :gn, :],
                                     mybir.ActivationFunctionType.Exp)
                sme = eppool.tile([P, GRP, 1], f32, tag="sme")
                nc.vector.reduce_sum(sme[:, :gn, :], ex[:, :gn, :],
                                     axis=mybir.AxisListType.X)
                lns = eppool.tile([P, GRP, 1], f32, tag="lns")
                nc.scalar.activation(lns[:, :gn, :], sme[:, :gn, :],
                                     mybir.ActivationFunctionType.Ln)
                cb = eppool.tile([P, GRP, 1], f32, tag="cb")
                nc.vector.tensor_tensor(cb[:, :gn, :], nmx[:, :gn, :],
                                        lns[:, :gn, :],
                                        op=mybir.AluOpType.subtract)
                oo = eppool.tile([P, GRP, F_OUT], f32, tag="oo")
                nc.vector.tensor_tensor(
                    oo[:, :gn, :], lgg[:, :gn, :],
                    cb[:, :gn, :].broadcast_to([P, gn, F_OUT]),
                    op=mybir.AluOpType.add)
                _store_batch([out], SHARD, oo, b0, bw, F_OUT)

    nc.compile()
    return nc


def _prepare(x, edge_row, edge_col, edge_val, W1, b1, W2, b2):
    C_gsu, off_gsu, GC_gs, moff, TC, MW, toff_t, meta, sdat = _preprocess(
        np.asarray(edge_row), np.asarray(edge_col), np.asarray(edge_val))
    nc = _build_program(C_gsu, off_gsu, GC_gs, moff, TC, MW, toff_t)

    x = np.asarray(x, np.float32)
    W1 = np.asarray(W1, np.float32)
    W2 = np.asarray(W2, np.float32)
    b1 = np.asarray(b1, np.float32)
    b2 = np.asarray(b2, np.float32)

    b1_np = b1.reshape(F_HID, 1).astype(np.float32)
    b2_np = np.broadcast_to(b2[None, :], (P, F_OUT)).copy().astype(np.float32)
    W1_np = W1.astype(BF16)
    W2_np = W2.astype(BF16)

    in_maps = []
    for c in range(NC):
        xk = x[c * SHARD:(c + 1) * SHARD]
        in_maps.append({
            "xT": np.ascontiguousarray(xk.T).astype(BF16),
            "W1b": W1_np, "W2b": W2_np,
            "b1c": b1_np, "b2bc": b2_np,
            "metad": meta[c],
            "sdat": sdat[c],
        })

    return nc, in_maps


def kernel(x, edge_row, edge_col, edge_val, W1, b1, W2, b2):
    nc, in_maps = _prepare(x, edge_row, edge_col, edge_val, W1, b1, W2, b2)
    res = run_bass_kernel_spmd(nc, in_maps, core_ids=list(range(NC)),
                               trace=False)
    return np.concatenate([res.results[c]["out"] for c in range(NC)], axis=0)
